# revision 4
# baseline (speedup 1.0000x reference)
"""Trainium2 Bass kernel for ConvolutionBlock: 1x1 pointwise (pad=1) + shared 3x3 conv.

Math: reference = valid3x3(zeropad1(pointwise(x))), with the SAME 3x3 filter
applied to every channel.  Since the depthwise 3x3 (shared filter) commutes
with the channel-mixing 1x1, we compute  y = pointwise(depthwise3x3(x)),
doing the 3x3 on 96 channels instead of 192.

Layout (per core, 4 batches):
  x  : [4, 96, 114, 114]  host-zero-padded, channels on SBUF partitions
  DW : for each 4-row output chunk, 9 accumulated matmuls with diagonal
       lhsT (k_uv * I_96), rhs = shifted window of x       -> PSUM [96,4,112]
  PW : 2 matmuls (C_OUT=192 split in two 96-halves), lhsT = W^T half
  All matmul operands bitcast to float32r (full-rate fp32 at N>=256).
"""

import numpy as np

from concourse import bacc, mybir
from concourse import tile
from concourse.bass_utils import run_bass_kernel_spmd

F32 = mybir.dt.float32
F32R = mybir.dt.float32r

B, C_IN, C_OUT, H, W = 32, 96, 192, 112, 112
N_CORES = 8
B_PER = B // N_CORES          # 4 batches per core
HP, WP = H + 2, W + 2         # 114, 114 padded
ROWS_PER_CHUNK = 4            # output rows per PSUM chunk  -> N = 448
N_CHUNKS = H // ROWS_PER_CHUNK  # 28
CHUNKS_PER_BLOCK = 7          # chunks accumulated in SBUF before out-DMA
N_BLOCKS = N_CHUNKS // CHUNKS_PER_BLOCK  # 4

_NC = None
LAST_RESULTS = None


def _build():
    nc = bacc.Bacc("TRN2", target_bir_lowering=False, debug=False,
                   num_devices=N_CORES)

    x_d = nc.dram_tensor("x", [B_PER, C_IN, HP, WP], F32R, kind="ExternalInput")
    wdiag_d = nc.dram_tensor("wdiag", [C_IN, 9, C_IN], F32R, kind="ExternalInput")
    wpcT_d = nc.dram_tensor("wpcT", [C_IN, 2, 96], F32R, kind="ExternalInput")
    y_d = nc.dram_tensor("y", [B_PER, C_OUT, H, W], F32, kind="ExternalOutput")

    with tile.TileContext(nc) as tc:
        with (
            tc.tile_pool(name="consts", bufs=1) as consts,
            tc.tile_pool(name="xin", bufs=2) as xin,
            tc.tile_pool(name="qs", bufs=4) as qsp,
            tc.tile_pool(name="ys", bufs=2) as ysp,
            tc.tile_pool(name="qp", bufs=2, space="PSUM") as qpp,
            tc.tile_pool(name="yp", bufs=4, space="PSUM") as ypp,
        ):
            w_sb = consts.tile([C_IN, 9, C_IN], F32R)
            nc.sync.dma_start(w_sb[:], wdiag_d[:])
            wpc_sb = consts.tile([C_IN, 2, 96], F32R)
            nc.sync.dma_start(wpc_sb[:], wpcT_d[:])

            copy_ctr = 0  # alternate PSUM-evac copies between ACT and DVE

            for b in range(B_PER):
                xb = xin.tile([C_IN, HP, WP], F32R)
                nc.sync.dma_start(xb[:], x_d[b])

                ys = [None, None]
                for ib in range(N_CHUNKS):
                    i0 = ib * ROWS_PER_CHUNK
                    slot = ib % CHUNKS_PER_BLOCK

                    # ---- depthwise 3x3: 9 accumulated diagonal matmuls ----
                    qp = qpp.tile([C_IN, ROWS_PER_CHUNK, W], F32)
                    t = 0
                    for u in range(3):
                        for v in range(3):
                            rhs = xb[:, i0 + u:i0 + u + ROWS_PER_CHUNK,
                                     v:v + W]
                            nc.tensor.matmul(
                                qp[:],
                                w_sb[:, t, :],
                                rhs,
                                start=(t == 0),
                                stop=(t == 8),
                            )
                            t += 1

                    # ---- evacuate DW result to SBUF ----
                    qs = qsp.tile([C_IN, ROWS_PER_CHUNK, W], F32R)
                    if copy_ctr % 2 == 0:
                        nc.scalar.copy(qs[:], qp[:])
                    else:
                        nc.vector.tensor_copy(qs[:], qp[:])
                    copy_ctr += 1

                    # ---- pointwise 1x1: two 96-output halves ----
                    for mt in range(2):
                        if slot == 0:
                            ys[mt] = ysp.tile(
                                [96, CHUNKS_PER_BLOCK * ROWS_PER_CHUNK, W],
                                F32, name=f"ys{mt}", tag=f"ys{mt}")
                        yp = ypp.tile([96, ROWS_PER_CHUNK, W], F32)
                        nc.tensor.matmul(
                            yp[:],
                            wpc_sb[:, mt, :],
                            qs[:],
                            start=True,
                            stop=True,
                        )
                        dst = ys[mt][:, slot * ROWS_PER_CHUNK:
                                     (slot + 1) * ROWS_PER_CHUNK, :]
                        if copy_ctr % 2 == 0:
                            nc.scalar.copy(dst, yp[:])
                        else:
                            nc.vector.tensor_copy(dst, yp[:])
                        copy_ctr += 1

                        if slot == CHUNKS_PER_BLOCK - 1:
                            blk = ib // CHUNKS_PER_BLOCK
                            r0 = blk * CHUNKS_PER_BLOCK * ROWS_PER_CHUNK
                            nc.sync.dma_start(
                                y_d[b, mt * 96:(mt + 1) * 96,
                                    r0:r0 + CHUNKS_PER_BLOCK * ROWS_PER_CHUNK,
                                    :],
                                ys[mt][:],
                            )

    nc.compile()
    return nc


def _prep_inputs(x, w_pc, w_dc):
    x = np.asarray(x, dtype=np.float32)
    k3 = np.asarray(w_dc, dtype=np.float32).reshape(3, 3)
    Wm = np.asarray(w_pc, dtype=np.float32).reshape(C_OUT, C_IN)

    xpad = np.zeros((B, C_IN, HP, WP), dtype=np.float32)
    xpad[:, :, 1:1 + H, 1:1 + W] = x

    wdiag = np.zeros((C_IN, 9, C_IN), dtype=np.float32)
    idx = np.arange(C_IN)
    wdiag[idx, :, idx] = k3.reshape(9)

    wpcT = np.ascontiguousarray(Wm.T.reshape(C_IN, 2, 96))
    return xpad, wdiag, wpcT


def kernel(x, w_pc, w_dc, _trace=False):
    global _NC, LAST_RESULTS
    if _NC is None:
        _NC = _build()

    xpad, wdiag, wpcT = _prep_inputs(x, w_pc, w_dc)
    in_maps = [
        {"x": np.ascontiguousarray(xpad[i * B_PER:(i + 1) * B_PER]),
         "wdiag": wdiag, "wpcT": wpcT}
        for i in range(N_CORES)
    ]
    res = run_bass_kernel_spmd(_NC, in_maps, list(range(N_CORES)),
                               trace=_trace)
    LAST_RESULTS = res
    y = np.concatenate([res.results[i]["y"] for i in range(N_CORES)], axis=0)
    return np.asarray(y, dtype=np.float32)
